# revision 11
# baseline (speedup 1.0000x reference)
"""HardBinaryConv Trainium2 kernel.

Computes y = conv2d(sign(x), sign(w)) for x [32,256,56,56] f32, w flat
[256*256*3*3, 1] f32, 3x3 kernel, stride 1, pad 1 (the STE forward pass of
reference.py).

Strategy: data-parallel over batch across 8 cores (4 images/core), weights
replicated. Per core: binarize x on the scalar engine (Sign) to fp8e4
(+-1/0 exact) into zero-padded 58x58 SBUF images, both 128-channel chunks
packed [128, 2, 3376] (16B-aligned stride for DoubleRow); binarize the
host-relaid-out weights to fp8. Conv = 9 accumulating fp8 DoubleRow
matmuls (256-channel contraction per pass, one per 3x3 tap) per PSUM tile
of [128 out-ch, 8 rows x 56 cols]; the rhs streams a strided [2, 8, 56]
window of the padded image, so horizontal taps are plain flat offsets and
padding columns are never computed.

The tensor engine (504 groups x 448 rows at fp8 DoubleRow rate) and the
DMA bus are nearly balanced, so the schedule keeps both saturated:
 - y is written as f16 (conv of +-1/0 values is an exact small integer;
   f16 holds integers exactly to 2048) and widened to f32 on the host.
 - w is uploaded as the high 2 bytes of each f32 (a pure byte-gather view
   = bf16 truncation; sign() of a truncated f32 is unchanged), split into
   two per-oc-chunk tensors so the first matmuls wait on half the bytes.
 - x arrives in 9/16/16/15-row chunks whose boundaries match the 8-row
   output blocks, so each sign() completion unlocks two more blocks.
 - image 0 alternates oc per block (halves the PE demand rate while the
   pipeline fills); image 3 runs oc-major so the tail ends in one small
   store; all stores are issued after every load is queued.
 - a bridge of tiny self-referential matmuls keeps the PE busy from t~0.5
   to the first real matmul so the p-state ramp is complete by then.

Since all matmul operands are exactly +-1/0 (sums of <=2304 of them are
exact integers in f32 PSUM and f16 output), the result is bit-exact vs
the reference.
"""

import numpy as np

import concourse.bass as bass
import concourse.bacc as bacc
import concourse.mybir as mybir
from concourse.tile import TileContext
from concourse.bass_utils import run_bass_kernel_spmd

N_CORES = 8
N_IMG = 4          # images per core
CIN = 256
COUT = 256
H = W = 56
WP = 58            # padded width
BASE = 2           # guard elements in front of the padded image
CSTRIDE = 3376     # per-c-chunk stride in the padded tile (16B aligned for fp8)
BLK = 8            # output rows per PSUM tile
NBLK = 7           # 56 / 8
NSPAN = BLK * WP   # 464 <= 512 (one PSUM bank in f32)

ROWCHUNKS = [(0, 9), (9, 16), (25, 16), (41, 15)]  # block b needs rows <= 8b+8

TRACE = False          # set by test.py to get a profile
LAST_RESULTS = None    # BassKernelResults of the last run (when TRACE)

W_BF16 = True          # upload weights as truncated-f32 (bf16 byte view)
X_BF16 = False         # upload x as truncated-f32 (bf16 byte view)
Y_F16 = True           # store y as f16 (exact for this op), widen on host
N_BRIDGE = 270         # warm-up matmuls bridging t~0.5us .. first real matmul

_cache = {}


def _build_nc():
    nc = bacc.Bacc("TRN2", num_devices=N_CORES)
    f32 = mybir.dt.float32
    bdt = mybir.dt.float8e4
    xdt = mybir.dt.bfloat16 if X_BF16 else f32
    wdt = mybir.dt.bfloat16 if W_BF16 else f32
    ydt = mybir.dt.float16 if Y_F16 else f32

    x_t = nc.dram_tensor("x", [N_IMG, CIN, H, W], xdt, kind="ExternalInput")
    # host-prepped weight layout: [o-chunk, c%128, c//128, tap(3*dh+dw), o]
    w_t = nc.dram_tensor("w", [2, 128, 2, 9, 128], wdt, kind="ExternalInput")
    y_t = nc.dram_tensor("y", [N_IMG, COUT, H, W], ydt, kind="ExternalOutput")
    x_ap, w_ap, y_ap = x_t.ap(), w_t.ap(), y_t.ap()

    chunks = [(n, r0, nr) for n in range(N_IMG) for r0, nr in ROWCHUNKS]

    with TileContext(nc) as tc:
        with (
            tc.tile_pool(name="persist", bufs=1) as persist,
            tc.tile_pool(name="stq", bufs=6) as stq,
            tc.tile_pool(name="outp", bufs=2 * N_IMG) as outp,
            tc.tile_pool(name="psum", bufs=7, space="PSUM") as psump,
            tc.tile_pool(name="psbr", bufs=1, space="PSUM") as psbr,
        ):
            # --- PE p-state warm-up bridge: tiny matmuls on a zeroed tile ---
            dz = persist.tile([128, 2, 192], bdt, name="dz")
            nc.gpsimd.memset(dz, 0.0)
            psd = psbr.tile([128, 64], f32, name="psd")
            for _ in range(N_BRIDGE):
                nc.tensor.matmul(
                    psd,
                    dz[:, :, 0:128],
                    dz[:, :, 128:192],
                    start=True,
                    stop=True,
                    perf_mode=mybir.MatmulPerfMode.DoubleRow,
                )

            # --- padded binarized images: [128, cc=2, 3376] ---
            xp = []
            for n in range(N_IMG):
                p = persist.tile([128, 2, CSTRIDE], bdt, name=f"xp_{n}")
                # zero guard/border cells: front guard + top row + row1-col0;
                # row56-col57 + bottom row + back guard; and the interleaved
                # (col57, next-row col0) pairs of interior rows
                nc.gpsimd.memset(p[:, :, 0 : BASE + WP + 1], 0.0)
                nc.gpsimd.memset(p[:, :, BASE + 57 * WP - 1 : CSTRIDE], 0.0)
                pairs = p[:, :, BASE + WP + 57 : BASE + 56 * WP + 57]
                pairs = pairs.rearrange("p k (r c) -> p k r c", c=WP)[:, :, :, 0:2]
                nc.gpsimd.memset(pairs, 0.0)
                xp.append(p)

            def load_chunk(n, r0, nr):
                src = x_ap[n].rearrange("(k p) h w -> p k h w", p=128)
                xf = stq.tile([128, 2, 16, W], xdt, name="xf", tag="xf")
                nc.sync.dma_start(xf[:, :, 0:nr], src[:, :, r0 : r0 + nr])
                return xf

            def sign_chunk(n, r0, nr, xf):
                interior = xp[n][:, :, BASE + WP + 1 : BASE + WP + 1 + H * WP]
                interior = interior.rearrange("p k (r c) -> p k r c", c=WP)[
                    :, :, :, 0:W
                ]
                nc.scalar.sign(interior[:, :, r0 : r0 + nr], xf[:, :, 0:nr])

            # lead-in critical chain: the first x chunk loads first (its sign
            # runs while the weights stream in); each per-oc weight tensor
            # arrives and is signed in two tap-halves so the first matmul of
            # a group starts as soon as its early taps are binarized
            wf = [
                persist.tile([128, 2, 9, 128], wdt, name=f"wf{oc}")
                for oc in range(2)
            ]
            wb = [
                persist.tile([128, 2, 9, 128], bdt, name=f"wb{oc}")
                for oc in range(2)
            ]
            xf0 = load_chunk(*chunks[0])
            sign_chunk(*chunks[0], xf0)
            for oc in range(2):
                for taps in (slice(0, 5), slice(5, 9)):
                    nc.sync.dma_start(wf[oc][:, :, taps], w_ap[oc][:, :, taps])
                    nc.scalar.sign(wb[oc][:, :, taps], wf[oc][:, :, taps])
            for ch in chunks[1:]:
                sign_chunk(*ch, load_chunk(*ch))

            # --- conv: per (img, block, oc): 9 accumulating tap matmuls ---
            def conv_group(n, b, oc, ob):
                ps = psump.tile([128, BLK, W], f32, name="ps", tag="ps")
                for dh in range(3):
                    for dw in range(3):
                        t = 3 * dh + dw
                        s = BASE + (BLK * b + dh) * WP + dw - 1
                        rhs = xp[n][:, :, s : s + NSPAN].rearrange(
                            "p k (r c) -> p k r c", c=WP
                        )[..., 1:57]
                        nc.tensor.matmul(
                            ps,
                            wb[oc][:, :, t],
                            rhs,
                            start=(t == 0),
                            stop=(t == 8),
                            perf_mode=mybir.MatmulPerfMode.DoubleRow,
                        )
                nc.vector.tensor_copy(out=ob[:, BLK * b : BLK * (b + 1), :], in_=ps)

            # stores deferred past all loads; rows split [0,24)/[24,56) so the
            # only non-overlappable store (the very last) is small
            stores = []
            for n in range(N_IMG):
                ob = [
                    outp.tile([128, H, W], ydt, name="ob", tag="ob")
                    for _ in range(2)
                ]
                if n < N_IMG - 1:
                    # oc alternates per block: halves the PE demand rate on
                    # not-yet-signed rows while the pipeline fills
                    for b in range(NBLK):
                        for oc in range(2):
                            conv_group(n, b, oc, ob[oc])
                    order = [(0, 0), (1, 0), (0, 1), (1, 1)]
                else:
                    # oc-major: oc1 finishes last, alone, and its store is
                    # split finely so the non-overlappable tail is 8 rows
                    for oc in range(2):
                        for b in range(NBLK):
                            conv_group(n, b, oc, ob[oc])
                    order = [(0, 0), (0, 1), (1, 2), (1, 3), (1, 4)]
                parts = {
                    0: slice(0, 24),
                    1: slice(24, 56),
                    2: slice(0, 24),
                    3: slice(24, 48),
                    4: slice(48, 56),
                }
                for oc, part in order:
                    rows = parts[part]
                    stores.append(
                        (y_ap[n, oc * 128 : (oc + 1) * 128][:, rows], ob[oc][:, rows])
                    )
            for dst, src in stores:
                nc.sync.dma_start(dst, src)
    nc.compile()
    return nc


def _bf16_view(a: np.ndarray) -> np.ndarray:
    """High 2 bytes of each f32 (little-endian) as bfloat16 — a pure byte
    gather; no value arithmetic. sign(bf16_view(v)) == sign(v) for every
    normal f32."""
    import ml_dtypes

    a = np.ascontiguousarray(a, dtype=np.float32)
    hi = a.view(np.uint16).reshape(*a.shape, 2)[..., 1]
    return np.ascontiguousarray(hi).view(ml_dtypes.bfloat16)


def _prep_weights(weights: np.ndarray) -> np.ndarray:
    w = np.asarray(weights, dtype=np.float32).reshape(COUT, CIN, 3, 3)
    # [o, c, dh, dw] -> [o//128, c%128, c//128, tap, o%128]
    w = w.reshape(2, 128, 2, 128, 9)  # [o2, o, c2, c, tap]
    w = w.transpose(0, 3, 2, 4, 1)  # [o2, c, c2, tap, o]
    w = np.ascontiguousarray(w)
    return _bf16_view(w) if W_BF16 else w


def kernel(x: np.ndarray, weights: np.ndarray) -> np.ndarray:
    global LAST_RESULTS
    if "nc" not in _cache:
        _cache["nc"] = _build_nc()
    nc = _cache["nc"]

    x = np.ascontiguousarray(np.asarray(x, dtype=np.float32))
    if X_BF16:
        x = _bf16_view(x)
    wprep = _prep_weights(weights)
    in_maps = [
        {"x": x[i * N_IMG : (i + 1) * N_IMG], "w": wprep} for i in range(N_CORES)
    ]
    res = run_bass_kernel_spmd(
        nc, in_maps, core_ids=list(range(N_CORES)), trace=TRACE
    )
    LAST_RESULTS = res
    return np.concatenate([r["y"] for r in res.results], axis=0).astype(
        np.float32
    )


# revision 12
# speedup vs baseline: 1.0155x; 1.0155x over previous
"""HardBinaryConv Trainium2 kernel.

Computes y = conv2d(sign(x), sign(w)) for x [32,256,56,56] f32, w flat
[256*256*3*3, 1] f32, 3x3 kernel, stride 1, pad 1 (the STE forward pass of
reference.py).

Strategy: data-parallel over batch across 8 cores (4 images/core), weights
replicated. Per core: binarize x on the scalar engine (Sign) to fp8e4
(+-1/0 exact) into zero-padded 58x58 SBUF images, both 128-channel chunks
packed [128, 2, 3376] (16B-aligned stride for DoubleRow); binarize the
host-relaid-out weights to fp8. Conv = 9 accumulating fp8 DoubleRow
matmuls (256-channel contraction per pass, one per 3x3 tap) per PSUM tile
of [128 out-ch, 8 rows x 56 cols]; the rhs streams a strided [2, 8, 56]
window of the padded image, so horizontal taps are plain flat offsets and
padding columns are never computed.

The tensor engine (504 groups x 448 rows at fp8 DoubleRow rate) and the
DMA bus are nearly balanced, so the schedule keeps both saturated:
 - y is written as f16 (conv of +-1/0 values is an exact small integer;
   f16 holds integers exactly to 2048) and widened to f32 on the host.
 - w is uploaded as the high 2 bytes of each f32 (a pure byte-gather view
   = bf16 truncation; sign() of a truncated f32 is unchanged), split into
   two per-oc-chunk tensors so the first matmuls wait on half the bytes.
 - x arrives in 9/16/16/15-row chunks whose boundaries match the 8-row
   output blocks, so each sign() completion unlocks two more blocks.
 - image 0 alternates oc per block (halves the PE demand rate while the
   pipeline fills); image 3 runs oc-major so the tail ends in one small
   store; all stores are issued after every load is queued.
 - a bridge of tiny self-referential matmuls keeps the PE busy from t~0.5
   to the first real matmul so the p-state ramp is complete by then.

Since all matmul operands are exactly +-1/0 (sums of <=2304 of them are
exact integers in f32 PSUM and f16 output), the result is bit-exact vs
the reference.
"""

import numpy as np

import concourse.bass as bass
import concourse.bacc as bacc
import concourse.mybir as mybir
from concourse.tile import TileContext
from concourse.bass_utils import run_bass_kernel_spmd

N_CORES = 8
N_IMG = 4          # images per core
CIN = 256
COUT = 256
H = W = 56
WP = 58            # padded width
BASE = 2           # guard elements in front of the padded image
CSTRIDE = 3376     # per-c-chunk stride in the padded tile (16B aligned for fp8)
BLK = 8            # output rows per PSUM tile
NBLK = 7           # 56 / 8
NSPAN = BLK * WP   # 464 <= 512 (one PSUM bank in f32)

ROWCHUNKS = [(0, 9), (9, 16), (25, 16), (41, 15)]  # block b needs rows <= 8b+8

TRACE = False          # set by test.py to get a profile
LAST_RESULTS = None    # BassKernelResults of the last run (when TRACE)

W_BF16 = True          # upload weights as truncated-f32 (bf16 byte view)
X_BF16 = True          # upload x as truncated-f32 (bf16 byte view)
Y_F16 = True           # store y as f16 (exact for this op), widen on host
N_BRIDGE = 270         # warm-up matmuls bridging t~0.5us .. first real matmul

_cache = {}


def _build_nc():
    nc = bacc.Bacc("TRN2", num_devices=N_CORES)
    f32 = mybir.dt.float32
    bdt = mybir.dt.float8e4
    xdt = mybir.dt.bfloat16 if X_BF16 else f32
    wdt = mybir.dt.bfloat16 if W_BF16 else f32
    ydt = mybir.dt.float16 if Y_F16 else f32

    x_t = nc.dram_tensor("x", [N_IMG, CIN, H, W], xdt, kind="ExternalInput")
    # host-prepped weight layout: [o-chunk, c%128, c//128, tap(3*dh+dw), o]
    w_t = nc.dram_tensor("w", [2, 128, 2, 9, 128], wdt, kind="ExternalInput")
    y_t = nc.dram_tensor("y", [N_IMG, COUT, H, W], ydt, kind="ExternalOutput")
    x_ap, w_ap, y_ap = x_t.ap(), w_t.ap(), y_t.ap()

    chunks = [(n, r0, nr) for n in range(N_IMG) for r0, nr in ROWCHUNKS]

    with TileContext(nc) as tc:
        with (
            tc.tile_pool(name="persist", bufs=1) as persist,
            tc.tile_pool(name="stq", bufs=6) as stq,
            tc.tile_pool(name="outp", bufs=2 * N_IMG) as outp,
            tc.tile_pool(name="psum", bufs=7, space="PSUM") as psump,
            tc.tile_pool(name="psbr", bufs=1, space="PSUM") as psbr,
        ):
            # --- PE p-state warm-up bridge: tiny matmuls on a zeroed tile ---
            dz = persist.tile([128, 2, 192], bdt, name="dz")
            nc.gpsimd.memset(dz, 0.0)
            psd = psbr.tile([128, 64], f32, name="psd")
            for _ in range(N_BRIDGE):
                nc.tensor.matmul(
                    psd,
                    dz[:, :, 0:128],
                    dz[:, :, 128:192],
                    start=True,
                    stop=True,
                    perf_mode=mybir.MatmulPerfMode.DoubleRow,
                )

            # --- padded binarized images: [128, cc=2, 3376] ---
            xp = []
            for n in range(N_IMG):
                p = persist.tile([128, 2, CSTRIDE], bdt, name=f"xp_{n}")
                # zero guard/border cells: front guard + top row + row1-col0;
                # row56-col57 + bottom row + back guard; and the interleaved
                # (col57, next-row col0) pairs of interior rows
                nc.gpsimd.memset(p[:, :, 0 : BASE + WP + 1], 0.0)
                nc.gpsimd.memset(p[:, :, BASE + 57 * WP - 1 : CSTRIDE], 0.0)
                pairs = p[:, :, BASE + WP + 57 : BASE + 56 * WP + 57]
                pairs = pairs.rearrange("p k (r c) -> p k r c", c=WP)[:, :, :, 0:2]
                nc.gpsimd.memset(pairs, 0.0)
                xp.append(p)

            def load_chunk(n, r0, nr):
                src = x_ap[n].rearrange("(k p) h w -> p k h w", p=128)
                xf = stq.tile([128, 2, 16, W], xdt, name="xf", tag="xf")
                nc.sync.dma_start(xf[:, :, 0:nr], src[:, :, r0 : r0 + nr])
                return xf

            def sign_chunk(n, r0, nr, xf):
                interior = xp[n][:, :, BASE + WP + 1 : BASE + WP + 1 + H * WP]
                interior = interior.rearrange("p k (r c) -> p k r c", c=WP)[
                    :, :, :, 0:W
                ]
                nc.scalar.sign(interior[:, :, r0 : r0 + nr], xf[:, :, 0:nr])

            # lead-in critical chain: the first x chunk loads first (its sign
            # runs while the weights stream in); each per-oc weight tensor
            # arrives and is signed in two tap-halves so the first matmul of
            # a group starts as soon as its early taps are binarized
            wf = [
                persist.tile([128, 2, 9, 128], wdt, name=f"wf{oc}")
                for oc in range(2)
            ]
            wb = [
                persist.tile([128, 2, 9, 128], bdt, name=f"wb{oc}")
                for oc in range(2)
            ]
            xf0 = load_chunk(*chunks[0])
            sign_chunk(*chunks[0], xf0)
            for oc in range(2):
                for taps in (slice(0, 5), slice(5, 9)):
                    nc.sync.dma_start(wf[oc][:, :, taps], w_ap[oc][:, :, taps])
                    nc.scalar.sign(wb[oc][:, :, taps], wf[oc][:, :, taps])
            for ch in chunks[1:]:
                sign_chunk(*ch, load_chunk(*ch))

            # --- conv: per (img, block, oc): 9 accumulating tap matmuls ---
            def conv_group(n, b, oc, ob):
                ps = psump.tile([128, BLK, W], f32, name="ps", tag="ps")
                for dh in range(3):
                    for dw in range(3):
                        t = 3 * dh + dw
                        s = BASE + (BLK * b + dh) * WP + dw - 1
                        rhs = xp[n][:, :, s : s + NSPAN].rearrange(
                            "p k (r c) -> p k r c", c=WP
                        )[..., 1:57]
                        nc.tensor.matmul(
                            ps,
                            wb[oc][:, :, t],
                            rhs,
                            start=(t == 0),
                            stop=(t == 8),
                            perf_mode=mybir.MatmulPerfMode.DoubleRow,
                        )
                nc.vector.tensor_copy(out=ob[:, BLK * b : BLK * (b + 1), :], in_=ps)

            # stores deferred past all loads; rows split [0,24)/[24,56) so the
            # only non-overlappable store (the very last) is small
            stores = []
            for n in range(N_IMG):
                ob = [
                    outp.tile([128, H, W], ydt, name="ob", tag="ob")
                    for _ in range(2)
                ]
                if n < N_IMG - 1:
                    # oc alternates per block: halves the PE demand rate on
                    # not-yet-signed rows while the pipeline fills
                    for b in range(NBLK):
                        for oc in range(2):
                            conv_group(n, b, oc, ob[oc])
                    order = [(0, 0), (1, 0), (0, 1), (1, 1)]
                else:
                    # oc-major: oc1 finishes last, alone, and its store is
                    # split finely so the non-overlappable tail is 8 rows
                    for oc in range(2):
                        for b in range(NBLK):
                            conv_group(n, b, oc, ob[oc])
                    order = [(0, 0), (0, 1), (1, 2), (1, 3), (1, 4)]
                parts = {
                    0: slice(0, 24),
                    1: slice(24, 56),
                    2: slice(0, 24),
                    3: slice(24, 48),
                    4: slice(48, 56),
                }
                for oc, part in order:
                    rows = parts[part]
                    stores.append(
                        (y_ap[n, oc * 128 : (oc + 1) * 128][:, rows], ob[oc][:, rows])
                    )
            for dst, src in stores:
                nc.sync.dma_start(dst, src)
    nc.compile()
    return nc


def _bf16_view(a: np.ndarray) -> np.ndarray:
    """High 2 bytes of each f32 (little-endian) as bfloat16 — a pure byte
    gather; no value arithmetic. sign(bf16_view(v)) == sign(v) for every
    normal f32."""
    import ml_dtypes

    a = np.ascontiguousarray(a, dtype=np.float32)
    hi = a.view(np.uint16).reshape(*a.shape, 2)[..., 1]
    return np.ascontiguousarray(hi).view(ml_dtypes.bfloat16)


def _prep_weights(weights: np.ndarray) -> np.ndarray:
    w = np.asarray(weights, dtype=np.float32).reshape(COUT, CIN, 3, 3)
    # [o, c, dh, dw] -> [o//128, c%128, c//128, tap, o%128]
    w = w.reshape(2, 128, 2, 128, 9)  # [o2, o, c2, c, tap]
    w = w.transpose(0, 3, 2, 4, 1)  # [o2, c, c2, tap, o]
    w = np.ascontiguousarray(w)
    return _bf16_view(w) if W_BF16 else w


def kernel(x: np.ndarray, weights: np.ndarray) -> np.ndarray:
    global LAST_RESULTS
    if "nc" not in _cache:
        _cache["nc"] = _build_nc()
    nc = _cache["nc"]

    x = np.ascontiguousarray(np.asarray(x, dtype=np.float32))
    if X_BF16:
        x = _bf16_view(x)
    wprep = _prep_weights(weights)
    in_maps = [
        {"x": x[i * N_IMG : (i + 1) * N_IMG], "w": wprep} for i in range(N_CORES)
    ]
    res = run_bass_kernel_spmd(
        nc, in_maps, core_ids=list(range(N_CORES)), trace=TRACE
    )
    LAST_RESULTS = res
    return np.concatenate([r["y"] for r in res.results], axis=0).astype(
        np.float32
    )
